# revision 1
# baseline (speedup 1.0000x reference)
"""Trainium2 Bass kernel for CapsuleLayer (nn_CapsuleLayer_45552423142009).

Computes, for x[B,768]:
  u = squash(x @ Wp + bp)            # [B, 8, 16]  (squash over last dim)
  u_hat[b,p,c,:] = u[b,p,:] @ W[p,c] # [B, 8, 5, 16]
  3 iterations of dynamic routing -> v [B, 5, 16]

Strategy: pure data-parallel over 8 NeuronCores (batch sharded 16384/core).
On-chip layout is "transposed": features on partitions, batch on the free
dim (512-wide tiles).  PE does transposes of x, the two big matmuls and all
broadcast / segment-sum reductions (via 0/1 selector matrices, fp32r at
1 cycle/row).  DVE/ACT/GPSIMD do the elementwise work.
"""

import sys
import numpy as np

sys.path.insert(0, "/opt/trn_rl_repo")

from concourse import bass, bacc, mybir  # noqa: E402
from concourse import tile  # noqa: E402
from concourse.bass_utils import run_bass_kernel_spmd  # noqa: E402
from concourse.alu_op_type import AluOpType  # noqa: E402

F32 = mybir.dt.float32
F32R = mybir.dt.float32r
AF = mybir.ActivationFunctionType

B = 131072
D = 768
P = 8
PD = 16
C = 5
CD = 16
NCORES = 8
BC = B // NCORES          # 16384 batch rows per core
NB = 512                  # batch columns per tile
NT = BC // NB             # 32 tiles

# selector blob column offsets
SEL_SSEL8 = 0      # [128, 8]   sum o-groups of 16 -> p
SEL_PSEL16 = 8     # [128, 16]  0.2 * (sum over p at fixed j)
SEL_PSEL8 = 24     # [128, 8]   sum over j at fixed p
SEL_IDENT = 32     # [128, 128] identity
SEL_TILE8 = 160    # [16, 128]  broadcast j -> (p, j)
SEL_SBC = 288      # [8, 128]   broadcast p -> (p, o)
SEL_JSEL = 416     # [80, 5]    sum over j at fixed c
SEL_JBC = 421      # [5, 80]    broadcast c -> (c, j)
SEL_CSEL = 501     # [40, 8]    sum over c at fixed p   (logits layout (c,p))
SEL_CBC = 509      # [8, 40]    broadcast p -> (c, p)
SEL_BSEL = 549     # [40, 640]  5 x [40,128]: broadcast (c,p) -> (p,i) for class c
SEL_ESEL = 1190    # [16, 40]   5 x [16,8]: col c ones (vsq accumulate)
SEL_GBC = 1230     # [8, 80]    5 x [8,16]: row c ones (g -> j-bcast, class c)
SEL_GBC40 = 1310   # [8, 40]    [c', (c,p)] = d_c'c (g -> (c,p) bcast)
SEL_ASEL = 1350    # [128, 200] 5 x [128,40]: [(p,j),(c',p')] = d_pp' d_c'c
SEL_W = 1550
CST_W = SEL_W + 768 + 640 + 80 + 1


def _r(ap):
    return ap.bitcast(F32R)


def build_selectors() -> np.ndarray:
    sel = np.zeros((128, SEL_W), dtype=np.float32)
    for m in range(128):
        sel[m, SEL_SSEL8 + m // 16] = 1.0                      # Ssel8
    for p in range(P):
        for j in range(PD):
            sel[p * 16 + j, SEL_PSEL16 + j] = 0.2              # Psel16 (x0.2)
            sel[p * 16 + j, SEL_PSEL8 + p] = 1.0               # Psel8
    sel[:, SEL_IDENT:SEL_IDENT + 128] = np.eye(128, dtype=np.float32)
    for j in range(16):
        for p in range(P):
            sel[j, SEL_TILE8 + p * 16 + j] = 1.0               # Tile8
    for p in range(P):
        sel[p, SEL_SBC + p * 16:SEL_SBC + (p + 1) * 16] = 1.0  # Sbc
    for c in range(C):
        for j in range(CD):
            sel[c * 16 + j, SEL_JSEL + c] = 1.0                # Jsel
            sel[c, SEL_JBC + c * 16 + j] = 1.0                 # Jbc
    for c in range(C):
        for p in range(P):
            sel[c * 8 + p, SEL_CSEL + p] = 1.0                 # Csel
            sel[p, SEL_CBC + c * 8 + p] = 1.0                  # Cbc
            # Bsel_c: [(c',p), (p',i)] = d_cc' d_pp'
            sel[c * 8 + p, SEL_BSEL + c * 128 + p * 16:
                SEL_BSEL + c * 128 + (p + 1) * 16] = 1.0
    for c in range(C):
        for j in range(CD):
            sel[j, SEL_ESEL + c * 8 + c] = 1.0                 # Esel_c col c
            sel[c, SEL_GBC + c * 16 + j] = 1.0                 # Gbc_c row c
    for c in range(C):
        for p in range(P):
            sel[c, SEL_GBC40 + c * 8 + p] = 1.0                # Gbc40
    for c in range(C):
        for p in range(P):
            for j in range(CD):
                # Asel_c: [(p,j), (c',p')] = d_pp' d_c'c
                sel[p * 16 + j, SEL_ASEL + c * 40 + c * 8 + p] = 1.0
    return sel


def build_nc(nt: int = NT) -> bass.Bass:
    bc = nt * NB
    nc = bacc.Bacc(None)

    x_d = nc.declare_dram_parameter("xc", [bc, D], F32R, isOutput=False)
    cst_d = nc.declare_dram_parameter("cst", [128, CST_W], F32R, isOutput=False)
    v_d = nc.declare_dram_parameter("vout", [bc, C * CD], F32, isOutput=True)

    with tile.TileContext(nc) as tc, nc.allow_low_precision(reason="float32r matmul inputs"):
        with (
            tc.sbuf_pool(name="const", bufs=1) as cpool,
            tc.sbuf_pool(name="xin", bufs=2) as xpool,
            tc.sbuf_pool(name="xt", bufs=2) as xtpool,
            tc.sbuf_pool(name="mid", bufs=2) as mpool,
            tc.sbuf_pool(name="uh", bufs=2) as uhpool,
            tc.sbuf_pool(name="rt", bufs=2) as rtpool,
            tc.sbuf_pool(name="sm", bufs=3) as smpool,
            tc.psum_pool(name="pxt", bufs=1) as pxt,
            tc.psum_pool(name="puh", bufs=2) as puhp,
            tc.psum_pool(name="pbc", bufs=2) as pbcp,
            tc.psum_pool(name="pmid", bufs=1) as pmidp,
            tc.psum_pool(name="psm", bufs=2) as psmp,
        ):
            # ---- load constants (one DMA), then stage through DVE so every
            # consumer depends on the DVE semaphore (merges with data deps;
            # walrus allows only ~2 distinct sync waits per instruction) ----
            cst0 = cpool.tile([128, CST_W], F32R)
            nc.sync.dma_start(out=cst0[:], in_=cst_d[:])
            cst = cpool.tile([128, CST_W], F32R)
            nc.vector.tensor_copy(cst[:], cst0[:])
            sel_sb = cst[:, 0:SEL_W]
            wp_sb = cst[:, SEL_W:SEL_W + 768]
            wbd_sb = cst[:, SEL_W + 768:SEL_W + 1408]
            wflat_sb = cst[:, SEL_W + 1408:SEL_W + 1488]
            bp_sb = cst[:, SEL_W + 1488:SEL_W + 1489].bitcast(F32)

            ident = sel_sb[:, SEL_IDENT:SEL_IDENT + 128]

            for it in range(nt):
                # ---- load x tile [512, 768] as 4 x [128, 768] ----
                x_sb = xpool.tile([128, 4, 768], F32R, tag="xin")
                src = x_d[it * NB:(it + 1) * NB, :].rearrange(
                    "(q p) d -> p q d", p=128)
                nc.sync.dma_start(out=x_sb[:], in_=src)

                # ---- transpose x -> xT chunks [128(d), 512(b)] x 6 ----
                xT = xtpool.tile([128, 6, NB], F32R, tag="xt")
                for k in range(6):
                    pt = pxt.tile([128, NB], F32R, tag="pxt")
                    for q in range(4):
                        nc.tensor.transpose(
                            _r(pt[:, q * 128:(q + 1) * 128]),
                            _r(x_sb[:, q, k * 128:(k + 1) * 128]),
                            _r(ident),
                        )
                    if k % 2 == 0:
                        nc.vector.tensor_copy(xT[:, k, :], pt[:])
                    else:
                        nc.scalar.copy(xT[:, k, :], pt[:])

                # ---- mm1: u_pre[(p,o), b] = Wp^T x^T  (+bias via ACT) ----
                pu = pmidp.tile([128, NB], F32, tag="pmid")
                for k in range(6):
                    nc.tensor.matmul(
                        pu[:], _r(wp_sb[:, k * 128:(k + 1) * 128]),
                        _r(xT[:, k, :]), start=(k == 0), stop=(k == 5))
                u_pre = mpool.tile([128, NB], F32, tag="mid")
                nc.scalar.activation(u_pre[:], pu[:], AF.Identity,
                                     bias=bp_sb[:], scale=1.0)

                # ---- squash factor f[p, b] ----
                usq = mpool.tile([128, NB], F32R, tag="mid2")
                nc.gpsimd.tensor_mul(usq[:], u_pre[:], u_pre[:])
                psq = psmp.tile([8, NB], F32, tag="psm")
                nc.tensor.matmul(psq[:], _r(sel_sb[:, SEL_SSEL8:SEL_SSEL8 + 8]),
                                 _r(usq[:]), start=True, stop=True)
                srt = smpool.tile([8, NB], F32, tag="sm")
                nc.scalar.sqrt(srt[:], psq[:])
                sq1 = smpool.tile([8, NB], F32, tag="sm")
                nc.scalar.add(sq1[:], psq[:], 1.0)
                den = smpool.tile([8, NB], F32, tag="sm")
                # den = (srt + 1e-8) * sq1
                nc.vector.scalar_tensor_tensor(
                    den[:], srt[:], 1e-8, sq1[:],
                    op0=AluOpType.add, op1=AluOpType.mult)
                rden = smpool.tile([8, NB], F32, tag="sm")
                nc.vector.reciprocal(rden[:], den[:])
                fz = smpool.tile([8, NB], F32R, tag="sm")
                nc.vector.tensor_mul(fz[:], psq[:], rden[:])
                pfb = pbcp.tile([128, NB], F32, tag="pbc")
                nc.tensor.matmul(pfb[:], _r(sel_sb[:8, SEL_SBC:SEL_SBC + 128]),
                                 _r(fz[:]), start=True, stop=True)
                u = mpool.tile([128, NB], F32R, tag="mid3")
                nc.vector.tensor_mul(u[:], u_pre[:], pfb[:])

                # ---- u_hat_c = Wbd_c^T u   (5 psum banks -> sbuf) ----
                uh = []
                for c in range(C):
                    puh = puhp.tile([128, NB], F32, tag="puh")
                    nc.tensor.matmul(
                        puh[:], _r(wbd_sb[:, c * 128:(c + 1) * 128]),
                        _r(u[:]), start=True, stop=True)
                    uhc = uhpool.tile([128, NB], F32R, tag=f"uh{c}")
                    if c % 2 == 0:
                        nc.scalar.copy(uhc[:], puh[:])
                    else:
                        nc.vector.tensor_copy(uhc[:], puh[:])
                    uh.append(uhc)

                # ---- routing ----
                logit = None
                v5 = None
                for itr in range(3):
                    if itr > 0:
                        e = rtpool.tile([40, NB], F32R, tag="rt_e")
                        nc.scalar.activation(e[:], logit[:], AF.Exp)
                        pden = psmp.tile([8, NB], F32, tag="psm")
                        nc.tensor.matmul(
                            pden[:], _r(sel_sb[:40, SEL_CSEL:SEL_CSEL + 8]),
                            _r(e[:]), start=True, stop=True)
                        rd = smpool.tile([8, NB], F32R, tag="sm")
                        nc.vector.reciprocal(rd[:], pden[:])
                        pdb = pbcp.tile([40, NB], F32, tag="pbc")
                        nc.tensor.matmul(
                            pdb[:], _r(sel_sb[:8, SEL_CBC:SEL_CBC + 40]),
                            _r(rd[:]), start=True, stop=True)
                        cn = rtpool.tile([40, NB], F32R, tag="rt_cn")
                        nc.vector.tensor_mul(cn[:], e[:], pdb[:])

                    # s[j, c, b] per class via matmul; copy to sbuf (rounded)
                    s_sb = rtpool.tile([16, 5, NB], F32R, tag="rt_s")
                    for c in range(C):
                        psc = psmp.tile([16, NB], F32, tag="psm")
                        if itr == 0:
                            nc.tensor.matmul(
                                psc[:],
                                _r(sel_sb[:, SEL_PSEL16:SEL_PSEL16 + 16]),
                                _r(uh[c][:]), start=True, stop=True)
                        else:
                            pcb = pbcp.tile([128, NB], F32, tag="pbc")
                            nc.tensor.matmul(
                                pcb[:],
                                _r(sel_sb[:40, SEL_BSEL + c * 128:
                                          SEL_BSEL + (c + 1) * 128]),
                                _r(cn[:]), start=True, stop=True)
                            t = rtpool.tile([128, NB], F32R, tag="rt_t")
                            nc.vector.tensor_mul(t[:], u[:], pcb[:])
                            nc.tensor.matmul(
                                psc[:],
                                _r(wflat_sb[:, c * 16:(c + 1) * 16]),
                                _r(t[:]), start=True, stop=True)
                        if c % 2 == 0:
                            nc.scalar.copy(s_sb[:, c, :], psc[:])
                        else:
                            nc.vector.tensor_copy(s_sb[:, c, :], psc[:])

                    # vsq[c, b] = sum_j s^2 via accumulating one-hot matmuls
                    ssq = rtpool.tile([16, 5, NB], F32R, tag="rt_ssq")
                    nc.gpsimd.tensor_mul(ssq[:], s_sb[:], s_sb[:])
                    pvq = psmp.tile([8, NB], F32, tag="psm")
                    for c in range(C):
                        nc.tensor.matmul(
                            pvq[:], _r(sel_sb[:16, SEL_ESEL + c * 8:
                                              SEL_ESEL + (c + 1) * 8]),
                            _r(ssq[:, c, :]), start=(c == 0), stop=(c == 4))
                    # g = vsq / ((1+vsq) (sqrt(vsq)+1e-8))
                    vsrt = smpool.tile([8, NB], F32, tag="sm")
                    nc.scalar.sqrt(vsrt[:], pvq[:])
                    vsq1 = smpool.tile([8, NB], F32, tag="sm")
                    nc.scalar.add(vsq1[:], pvq[:], 1.0)
                    vden = smpool.tile([8, NB], F32, tag="sm")
                    nc.vector.scalar_tensor_tensor(
                        vden[:], vsrt[:], 1e-8, vsq1[:],
                        op0=AluOpType.add, op1=AluOpType.mult)
                    rvd = smpool.tile([8, NB], F32, tag="sm")
                    nc.vector.reciprocal(rvd[:], vden[:])
                    g = smpool.tile([8, NB], F32R, tag="sm")
                    nc.vector.tensor_mul(g[:], pvq[:], rvd[:])

                    if itr < 2:
                        # agreement with v = g*s folded after the j-sum:
                        # atil[(c,p), b] = sum_j uh_c[(p,j),b] * s[j,c,b]
                        pat = pmidp.tile([40, NB], F32, tag="pmid")
                        for c in range(C):
                            pvb = pbcp.tile([128, NB], F32, tag="pbc")
                            nc.tensor.matmul(
                                pvb[:],
                                _r(sel_sb[:16, SEL_TILE8:SEL_TILE8 + 128]),
                                _r(s_sb[:, c, :]), start=True, stop=True)
                            pr = rtpool.tile([128, NB], F32R, tag="rt_pr")
                            nc.vector.tensor_mul(pr[:], uh[c][:], pvb[:])
                            nc.tensor.matmul(
                                pat[:],
                                _r(sel_sb[:, SEL_ASEL + c * 40:
                                          SEL_ASEL + (c + 1) * 40]),
                                _r(pr[:]), start=(c == 0), stop=(c == 4))
                        ats = rtpool.tile([40, NB], F32, tag="rt_ats")
                        nc.scalar.copy(ats[:], pat[:])
                        pg40 = psmp.tile([40, NB], F32, tag="psm")
                        nc.tensor.matmul(
                            pg40[:], _r(sel_sb[:8, SEL_GBC40:SEL_GBC40 + 40]),
                            _r(g[:]), start=True, stop=True)
                        if itr == 0:
                            logit = rtpool.tile([40, NB], F32, tag="rt_lg")
                            nc.vector.tensor_mul(logit[:], ats[:], pg40[:])
                        else:
                            a40 = rtpool.tile([40, NB], F32, tag="rt_a40")
                            nc.vector.tensor_mul(a40[:], ats[:], pg40[:])
                            lg2 = rtpool.tile([40, NB], F32, tag="rt_lg2")
                            nc.vector.tensor_add(lg2[:], logit[:], a40[:])
                            logit = lg2
                    else:
                        # final v[j, c, b] = s * g_bcast
                        v5 = rtpool.tile([16, 5, NB], F32R, tag="rt_v")
                        for c in range(C):
                            pgb = psmp.tile([16, NB], F32, tag="psm")
                            nc.tensor.matmul(
                                pgb[:], _r(sel_sb[:8, SEL_GBC + c * 16:
                                                  SEL_GBC + (c + 1) * 16]),
                                _r(g[:]), start=True, stop=True)
                            nc.vector.tensor_mul(
                                v5[:, c, :], s_sb[:, c, :], pgb[:])

                # ---- transpose v back to [b, (c,j)] and store ----
                vo = rtpool.tile([128, 4, 80], F32, tag="rt_vo")
                for q in range(4):
                    pvt = pbcp.tile([128, 80], F32R, tag="pbc")
                    for c in range(C):
                        nc.tensor.transpose(
                            _r(pvt[:, c * 16:(c + 1) * 16]),
                            _r(v5[:, c, q * 128:(q + 1) * 128]),
                            _r(sel_sb[:16, SEL_IDENT:SEL_IDENT + 16]))
                    if q % 2 == 0:
                        nc.scalar.copy(vo[:, q, :], pvt[:])
                    else:
                        nc.vector.tensor_copy(vo[:, q, :], pvt[:])
                dst = v_d[it * NB:(it + 1) * NB, :].rearrange(
                    "(q p) j -> p q j", p=128)
                nc.sync.dma_start(out=dst, in_=vo[:])

    nc.compile()
    return nc


_NC_CACHE: dict = {}


def _get_nc(nt: int) -> bass.Bass:
    if nt not in _NC_CACHE:
        _NC_CACHE[nt] = build_nc(nt)
    return _NC_CACHE[nt]


def _prep_weights(Wp, bp, W):
    Wp = np.asarray(Wp, np.float32)
    bp = np.asarray(bp, np.float32)
    W = np.asarray(W, np.float32)
    wp_flat = Wp.transpose(1, 0, 2).reshape(768, 128)          # [d, (p,o)]
    wp_h = np.ascontiguousarray(
        wp_flat.reshape(6, 128, 128).transpose(1, 0, 2).reshape(128, 768))
    wbd_h = np.zeros((128, 5, 128), np.float32)
    for p in range(P):
        wbd_h[p * 16:(p + 1) * 16, :, p * 16:(p + 1) * 16] = \
            W[p].transpose(1, 0, 2)                            # [i, c, j]
    wbd_h = np.ascontiguousarray(wbd_h.reshape(128, 640))
    wflat_h = np.ascontiguousarray(
        W.transpose(0, 2, 1, 3).reshape(128, 5 * 16))          # [(p,i), (c,j)]
    bp_h = np.ascontiguousarray(bp.reshape(128, 1))
    sel_h = build_selectors()
    return wp_h, wbd_h, wflat_h, bp_h, sel_h


def pack_consts(Wp, bp, W):
    wp_h, wbd_h, wflat_h, bp_h, sel_h = _prep_weights(Wp, bp, W)
    cst = np.concatenate([sel_h, wp_h, wbd_h, wflat_h, bp_h], axis=1)
    assert cst.shape == (128, CST_W), cst.shape
    return np.ascontiguousarray(cst)


def kernel(x, Wp, bp, W):
    x = np.asarray(x, np.float32)
    cst = pack_consts(Wp, bp, W)
    nc = _get_nc(NT)
    in_maps = [{"xc": np.ascontiguousarray(x[i * BC:(i + 1) * BC]), "cst": cst}
               for i in range(NCORES)]
    res = run_bass_kernel_spmd(nc, in_maps, list(range(NCORES)))
    out = np.concatenate([res.results[i]["vout"] for i in range(NCORES)], axis=0)
    return out.reshape(B, C, CD)



# revision 2
# speedup vs baseline: 1.0720x; 1.0720x over previous
"""Trainium2 Bass kernel for CapsuleLayer (nn_CapsuleLayer_45552423142009), v4.

Two-pass structure:
  Pass 1 (dense PE stream, keeps HAM at K=8/8): per tile, DMA x,
    PE-transpose, mm1, squash -> u tile parked in SBUF (32 x 1KB/part).
  Pass 2 (routing): logits live in PSUM the whole time -- agreement
    accumulates b via start=False matmuls, and the softmax log-sum-exp
    subtraction is a negated-selector matmul into the same bank
    (softmax is shift-invariant, so the residual per-p shift cancels).
  v = s*g is computed every iteration and agreements contract v
    directly (u_hat never materialized).
  ACT uses only {Ln, Exp, Square, Identity, Copy}: one table set.
"""

import sys
import numpy as np

sys.path.insert(0, "/opt/trn_rl_repo")

from concourse import bass, bacc, mybir  # noqa: E402
from concourse import tile  # noqa: E402
from concourse.bass_utils import run_bass_kernel_spmd  # noqa: E402
from concourse.alu_op_type import AluOpType  # noqa: E402

# Pin ACT to the one table set covering every function used here
# (Ln, Exp, Square, Identity, Copy); placement otherwise first-fits
# Ln->natural_log / Exp->exp_and_others and thrashes table loads.
from concourse import hw_specs as _hw_specs  # noqa: E402

_ORIG_GAT = _hw_specs.get_activation_tables
_KEEP_SET = "natural_log_exp_and_others"


def _pinned_tables(arch):
    tabs = _ORIG_GAT(arch)
    return {k: (v if k == _KEEP_SET else set()) for k, v in tabs.items()}


_hw_specs.get_activation_tables = _pinned_tables
bacc.get_activation_tables = _pinned_tables

F32 = mybir.dt.float32
F32R = mybir.dt.float32r
BF16 = mybir.dt.bfloat16
AF = mybir.ActivationFunctionType

B = 131072
D = 768
P = 8
PD = 16
C = 5
CD = 16
NCORES = 8
BC = B // NCORES          # 16384 batch rows per core
NB = 512                  # batch columns per tile
NT = BC // NB             # 32 tiles

# bf16 const blob column offsets
OFF_WP = 0                # [128, 768]  mm1 weights, 6 chunks of [128,128]
OFF_SSEL = 768            # [128, 8]    sum 16-groups -> p
OFF_SBC = 776             # [8, 128]    broadcast p -> (p,i)
OFF_M0 = 904              # [128, 80]   0.2*W flat: s0 = M0^T u
OFF_JSEL = 984            # [80, 5]     sum j at fixed c
OFF_G80 = 1029            # [5, 80]     broadcast c -> (c,j)
OFF_CSEL = 1109           # [40, 8]     sum c at fixed p
OFF_CBCN = 1117           # [8, 40]     NEGATED broadcast p -> (c,p)
OFF_WTA = 1157            # [80, 640]   5 x [80,128]: wv_c = WTa_c^T v
OFF_ASEL = 1797           # [128, 200]  5 x [128,40]: sum i -> (c,p)
OFF_BSEL = 1997           # [40, 640]   5 x [40,128]: bcast (c,p)->(p,i)
OFF_WF = 2637             # [128, 400]  5 x [128,80]: s_c = Wf_c^T t_c
OFF_EYE = 3037            # [128, 128]  bf16 identity (out transposes)
CB_W = OFF_EYE + 128
CF_W = 129                # f32 blob: ident(128) + bp(1)


def build_consts(Wp, bp, W):
    """Return (cstf [128,129] f32, cstb [128,CB_W] bf16)."""
    Wp = np.asarray(Wp, np.float32)
    bp = np.asarray(bp, np.float32)
    W = np.asarray(W, np.float32)

    cstf = np.zeros((128, CF_W), np.float32)
    cstf[:, 0:128] = np.eye(128, dtype=np.float32)
    cstf[:, 128] = bp.reshape(128)

    cb = np.zeros((128, CB_W), np.float32)
    wp_flat = Wp.transpose(1, 0, 2).reshape(D, 128)           # [d, (p,o)]
    for k in range(6):
        cb[:, OFF_WP + k * 128:OFF_WP + (k + 1) * 128] = \
            wp_flat[k * 128:(k + 1) * 128, :]
    for p in range(P):
        for i in range(PD):
            cb[p * 16 + i, OFF_SSEL + p] = 1.0
            cb[p, OFF_SBC + p * 16 + i] = 1.0
    cb[:, OFF_M0:OFF_M0 + 80] = 0.2 * W.transpose(0, 2, 1, 3).reshape(128, 80)
    for c in range(C):
        for j in range(CD):
            cb[c * 16 + j, OFF_JSEL + c] = 1.0
            cb[c, OFF_G80 + c * 16 + j] = 1.0
    for c in range(C):
        for p in range(P):
            cb[c * 8 + p, OFF_CSEL + p] = 1.0
            cb[p, OFF_CBCN + c * 8 + p] = -1.0
    for c in range(C):
        # WTa_c rows (c*16+j), cols (p*16+i) = W[p,c,i,j]
        cb[c * 16:(c + 1) * 16, OFF_WTA + c * 128:OFF_WTA + (c + 1) * 128] = \
            W[:, c].transpose(2, 0, 1).reshape(16, 128)
        for p in range(P):
            cb[p * 16:(p + 1) * 16, OFF_ASEL + c * 40 + c * 8 + p] = 1.0
            cb[c * 8 + p, OFF_BSEL + c * 128 + p * 16:
               OFF_BSEL + c * 128 + (p + 1) * 16] = 1.0
        cb[:, OFF_WF + c * 80 + c * 16:OFF_WF + c * 80 + (c + 1) * 16] = \
            W[:, c].reshape(128, 16)
    cb[:, OFF_EYE:OFF_EYE + 128] = np.eye(128, dtype=np.float32)

    import ml_dtypes
    cstb = cb.astype(ml_dtypes.bfloat16)
    return np.ascontiguousarray(cstf), np.ascontiguousarray(cstb)


def build_nc(nt: int = NT) -> bass.Bass:
    bc = nt * NB
    nc = bacc.Bacc(None)

    x_d = nc.declare_dram_parameter("xc", [bc, D], F32R, isOutput=False)
    cf_d = nc.declare_dram_parameter("cstf", [128, CF_W], F32R, isOutput=False)
    cb_d = nc.declare_dram_parameter("cstb", [128, CB_W], BF16, isOutput=False)
    v_d = nc.declare_dram_parameter("vout", [bc, C * CD], F32, isOutput=True)

    with tile.TileContext(nc) as tc, nc.allow_low_precision(reason="bf16 kernel"):
        with (
            tc.sbuf_pool(name="const", bufs=1) as cpool,
            tc.sbuf_pool(name="xin", bufs=3) as xpool,
            tc.sbuf_pool(name="xt", bufs=3) as xtpool,
            tc.sbuf_pool(name="mid", bufs=3) as mpool,
            tc.sbuf_pool(name="ubank", bufs=nt) as upool,
            tc.sbuf_pool(name="tmul", bufs=3) as tpool,
            tc.sbuf_pool(name="rt", bufs=3) as rtpool,
            tc.sbuf_pool(name="sm", bufs=2) as smpool,
            tc.sbuf_pool(name="out", bufs=3) as opool,
            tc.psum_pool(name="pA", bufs=2) as pA,          # pt/pu/ps/pvt
            tc.psum_pool(name="psm", bufs=2) as psm,        # small f32
            tc.psum_pool(name="pbc", bufs=2) as pbc,        # bcasts
            tc.psum_pool(name="pacc", bufs=2) as pacc,      # logits b
        ):
            # ---- constants: one DMA each, staged through DVE ----
            cf0 = cpool.tile([128, CF_W], F32R)
            nc.sync.dma_start(out=cf0[:], in_=cf_d[:])
            cf = cpool.tile([128, CF_W], F32R)
            nc.vector.tensor_copy(cf[:], cf0[:])
            cb0 = cpool.tile([128, CB_W], BF16)
            nc.sync.dma_start(out=cb0[:], in_=cb_d[:])
            cbs = cpool.tile([128, CB_W], BF16)
            nc.vector.tensor_copy(cbs[:], cb0[:])

            identf = cf[:, 0:128]
            bpf = cf[:, 128:129].bitcast(F32)

            def mm(out, lhsT, rhs, start=True, stop=True):
                nc.tensor.matmul(out, lhsT, rhs, start=start, stop=stop)

            def g_chain(pvq, rows, tag):
                """g = sqrt(s)/(1+s) = exp(0.5*ln(s) - ln(1+s))."""
                l1 = smpool.tile([rows, NB], F32, tag=f"l1{tag}")
                nc.scalar.activation(l1[:], pvq[:], AF.Ln)
                l2 = smpool.tile([rows, NB], F32, tag=f"l2{tag}")
                nc.scalar.activation(l2[:], pvq[:], AF.Ln, bias=1.0)
                gm = smpool.tile([rows, NB], F32, tag=f"gm{tag}")
                nc.vector.scalar_tensor_tensor(
                    gm[:], l1[:], 0.5, l2[:],
                    op0=AluOpType.mult, op1=AluOpType.subtract)
                g = smpool.tile([rows, NB], BF16, tag=f"g{tag}")
                nc.scalar.activation(g[:], gm[:], AF.Exp)
                return g

            # ---- PE warm-up burst: ~16 matmuls back to back (~4us) ----
            wt = pA.tile([128, NB], F32, tag="pA")
            for w in range(16):
                mm(wt[:], cbs[:, 0:128], cbs[:, 0:NB],
                   start=(w == 0), stop=(w == 15))

            u_tiles = []

            # ================= PASS 1: x -> u =================
            for it in range(nt):
                x_sb = xpool.tile([128, 4, D], F32R, tag="xin")
                src = x_d[it * NB:(it + 1) * NB, :].rearrange(
                    "(p q) d -> p q d", p=128)
                nc.sync.dma_start(out=x_sb[:], in_=src)

                xT = xtpool.tile([128, 6, NB], BF16, tag="xt")
                for k in range(6):
                    pt = pA.tile([128, NB], F32R, tag="pA")
                    for q in range(4):
                        nc.tensor.transpose(
                            pt[:, q * 128:(q + 1) * 128],
                            x_sb[:, q, k * 128:(k + 1) * 128],
                            identf)
                    nc.vector.tensor_copy(xT[:, k, :], pt[:])

                pu = pA.tile([128, NB], F32, tag="pA")
                for k in range(6):
                    mm(pu[:], cbs[:, OFF_WP + k * 128:OFF_WP + (k + 1) * 128],
                       xT[:, k, :], start=(k == 0), stop=(k == 5))
                u_pre = mpool.tile([128, NB], BF16, tag="upre")
                nc.scalar.activation(u_pre[:], pu[:], AF.Identity, bias=bpf)
                usq = mpool.tile([128, NB], BF16, tag="usq")
                nc.scalar.activation(usq[:], pu[:], AF.Square, bias=bpf)

                psq = psm.tile([8, NB], F32, tag="psm")
                mm(psq[:], cbs[:, OFF_SSEL:OFF_SSEL + 8], usq[:])
                f = g_chain(psq, 8, "f")
                pfb = pbc.tile([128, NB], F32, tag="pbc")
                mm(pfb[:], cbs[:8, OFF_SBC:OFF_SBC + 128], f[:])
                u = upool.tile([128, NB], BF16, tag="u")
                nc.vector.tensor_mul(u[:], u_pre[:], pfb[:])
                u_tiles.append(u)

            # ================= PASS 2: routing (tiles interleaved in pairs
            # so each engine always has an independent instruction ready) ===
            state = {}

            def emit_iter(it, itr):
                u = u_tiles[it]
                st = state.setdefault(it, {})
                pa = st.get("pa")

                ps = pA.tile([80, NB], F32, tag="pA")
                if itr == 0:
                    mm(ps[:], cbs[:, OFF_M0:OFF_M0 + 80], u[:])
                else:
                    e = rtpool.tile([40, NB], BF16, tag="rt_e")
                    nc.scalar.activation(e[:], pa[:], AF.Exp)
                    pden = psm.tile([8, NB], F32, tag="psm")
                    mm(pden[:], cbs[:40, OFF_CSEL:OFF_CSEL + 8], e[:])
                    lse = smpool.tile([8, NB], BF16, tag="lse")
                    nc.scalar.activation(lse[:], pden[:], AF.Ln)
                    mm(pa[:], cbs[:8, OFF_CBCN:OFF_CBCN + 40], lse[:],
                       start=False, stop=True)
                    cn = rtpool.tile([40, NB], BF16, tag="rt_cn")
                    nc.scalar.activation(cn[:], pa[:], AF.Exp)
                    t5 = tpool.tile([128, 5, NB], BF16, tag="t5")
                    for c in range(C):
                        pcb = pbc.tile([128, NB], F32, tag="pbc")
                        mm(pcb[:],
                           cbs[:40, OFF_BSEL + c * 128:
                               OFF_BSEL + (c + 1) * 128], cn[:])
                        nc.vector.tensor_mul(t5[:, c, :], u[:], pcb[:])
                    for c in range(C):
                        mm(ps[:], cbs[:, OFF_WF + c * 80:
                                      OFF_WF + (c + 1) * 80],
                           t5[:, c, :], start=(c == 0), stop=(c == 4))

                s_sb = rtpool.tile([80, NB], BF16, tag="rt_s")
                nc.scalar.copy(s_sb[:], ps[:])
                ssq = rtpool.tile([80, NB], BF16, tag="rt_ssq")
                nc.scalar.activation(ssq[:], ps[:], AF.Square)
                pvq = psm.tile([5, NB], F32, tag="psm")
                mm(pvq[:], cbs[:80, OFF_JSEL:OFF_JSEL + 5], ssq[:])
                g = g_chain(pvq, 5, "g")
                pg80 = pbc.tile([80, NB], F32, tag="pbc")
                mm(pg80[:], cbs[:5, OFF_G80:OFF_G80 + 80], g[:])
                v_sb = rtpool.tile([80, NB], BF16, tag="rt_v")
                nc.vector.tensor_mul(v_sb[:], s_sb[:], pg80[:])
                st["v"] = v_sb

                if itr < 2:
                    if itr == 0:
                        pa = pacc.tile([40, NB], F32, tag="pacc")
                        st["pa"] = pa
                    t5a = tpool.tile([128, 5, NB], BF16, tag="t5")
                    for c in range(C):
                        pwv = pbc.tile([128, NB], F32, tag="pbc")
                        mm(pwv[:],
                           cbs[:80, OFF_WTA + c * 128:
                               OFF_WTA + (c + 1) * 128], v_sb[:])
                        nc.vector.tensor_mul(t5a[:, c, :], u[:], pwv[:])
                    for c in range(C):
                        mm(pa[:], cbs[:, OFF_ASEL + c * 40:
                                      OFF_ASEL + (c + 1) * 40],
                           t5a[:, c, :],
                           start=(itr == 0 and c == 0), stop=(c == 4))

            def emit_out(it):
                v_sb = state[it]["v"]
                vo = opool.tile([128, 4, 80], F32, tag="vo")
                for q in range(4):
                    pvt = pA.tile([128, 80], BF16, tag="pA")
                    nc.tensor.transpose(
                        pvt[:, 0:80],
                        v_sb[:, q * 128:(q + 1) * 128],
                        cbs[:80, OFF_EYE:OFF_EYE + 80])
                    nc.vector.tensor_copy(vo[:, q, :], pvt[:, 0:80])
                dst = v_d[it * NB:(it + 1) * NB, :].rearrange(
                    "(p q) j -> p q j", p=128)
                nc.sync.dma_start(out=dst, in_=vo[:])

            for tp in range(0, nt, 2):
                pair = [t for t in (tp, tp + 1) if t < nt]
                for itr in range(3):
                    for t in pair:
                        emit_iter(t, itr)
                for t in pair:
                    emit_out(t)

    nc.compile()
    return nc


_NC_CACHE: dict = {}


def _get_nc(nt: int) -> bass.Bass:
    if nt not in _NC_CACHE:
        _NC_CACHE[nt] = build_nc(nt)
    return _NC_CACHE[nt]


def kernel(x, Wp, bp, W):
    x = np.asarray(x, np.float32)
    cstf, cstb = build_consts(Wp, bp, W)
    nc = _get_nc(NT)
    in_maps = [{"xc": np.ascontiguousarray(x[i * BC:(i + 1) * BC]),
                "cstf": cstf, "cstb": cstb}
               for i in range(NCORES)]
    res = run_bass_kernel_spmd(nc, in_maps, list(range(NCORES)))
    out = np.concatenate([res.results[i]["vout"] for i in range(NCORES)], axis=0)
    return out.reshape(B, C, CD)


# revision 3
# speedup vs baseline: 1.1097x; 1.0351x over previous
"""Trainium2 Bass kernel for CapsuleLayer (nn_CapsuleLayer_45552423142009).

Two-pass structure:
  Pass 1 (dense PE stream, keeps HAM at K=8/8): per tile, DMA x,
    PE-transpose, mm1, squash -> u tile parked in SBUF (32 x 1KB/part).
  Pass 2 (routing): logits live in PSUM the whole time -- agreement
    accumulates b via start=False matmuls, and the softmax log-sum-exp
    subtraction is a negated-selector matmul into the same bank
    (softmax is shift-invariant, so the residual per-p shift cancels).
  v = s*g is computed every iteration and agreements contract v
    directly (u_hat never materialized).
  ACT uses only {Ln, Exp, Square, Identity, Copy}: one table set.
"""

import sys
import numpy as np

sys.path.insert(0, "/opt/trn_rl_repo")

from concourse import bass, bacc, mybir  # noqa: E402
from concourse import tile  # noqa: E402
from concourse.bass_utils import run_bass_kernel_spmd  # noqa: E402
from concourse.alu_op_type import AluOpType  # noqa: E402

# Pin ACT to the one table set covering every function used here
# (Ln, Exp, Square, Identity, Copy); placement otherwise first-fits
# Ln->natural_log / Exp->exp_and_others and thrashes table loads.
from concourse import hw_specs as _hw_specs  # noqa: E402

_ORIG_GAT = _hw_specs.get_activation_tables
_KEEP_SET = "natural_log_exp_and_others"


def _pinned_tables(arch):
    tabs = _ORIG_GAT(arch)
    return {k: (v if k == _KEEP_SET else set()) for k, v in tabs.items()}


_hw_specs.get_activation_tables = _pinned_tables
bacc.get_activation_tables = _pinned_tables

F32 = mybir.dt.float32
F32R = mybir.dt.float32r
BF16 = mybir.dt.bfloat16
AF = mybir.ActivationFunctionType

B = 131072
D = 768
P = 8
PD = 16
C = 5
CD = 16
NCORES = 8
BC = B // NCORES          # 16384 batch rows per core
NB = 512                  # batch columns per tile
NT = BC // NB             # 32 tiles

# bf16 const blob column offsets
OFF_WP = 0                # [128, 768]  mm1 weights, 6 chunks of [128,128]
OFF_SSEL = 768            # [128, 8]    sum 16-groups -> p
OFF_SBC = 776             # [8, 128]    broadcast p -> (p,i)
OFF_M0 = 904              # [128, 80]   0.2*W flat: s0 = M0^T u
OFF_JSEL = 984            # [80, 5]     sum j at fixed c
OFF_G80 = 1029            # [5, 80]     broadcast c -> (c,j)
OFF_CSEL = 1109           # [40, 8]     sum c at fixed p
OFF_CBCN = 1117           # [8, 40]     NEGATED broadcast p -> (c,p)
OFF_WTA = 1157            # [80, 640]   5 x [80,128]: wv_c = WTa_c^T v
OFF_ASEL = 1797           # [128, 200]  5 x [128,40]: sum i -> (c,p)
OFF_BSEL = 1997           # [40, 640]   5 x [40,128]: bcast (c,p)->(p,i)
OFF_WF = 2637             # [128, 400]  5 x [128,80]: s_c = Wf_c^T t_c
OFF_EYE = 3037            # [128, 128]  bf16 identity (out transposes)
CB_W = OFF_EYE + 128
CF_W = 129                # f32 blob: ident(128) + bp(1)


def build_consts(Wp, bp, W):
    """Return (cstf [128,129] f32, cstb [128,CB_W] bf16)."""
    Wp = np.asarray(Wp, np.float32)
    bp = np.asarray(bp, np.float32)
    W = np.asarray(W, np.float32)

    cstf = np.zeros((128, CF_W), np.float32)
    cstf[:, 0:128] = np.eye(128, dtype=np.float32)
    cstf[:, 128] = bp.reshape(128)

    cb = np.zeros((128, CB_W), np.float32)
    wp_flat = Wp.transpose(1, 0, 2).reshape(D, 128)           # [d, (p,o)]
    for k in range(6):
        cb[:, OFF_WP + k * 128:OFF_WP + (k + 1) * 128] = \
            wp_flat[k * 128:(k + 1) * 128, :]
    for p in range(P):
        for i in range(PD):
            cb[p * 16 + i, OFF_SSEL + p] = 1.0
            cb[p, OFF_SBC + p * 16 + i] = 1.0
    cb[:, OFF_M0:OFF_M0 + 80] = 0.2 * W.transpose(0, 2, 1, 3).reshape(128, 80)
    for c in range(C):
        for j in range(CD):
            cb[c * 16 + j, OFF_JSEL + c] = 1.0
            cb[c, OFF_G80 + c * 16 + j] = 1.0
    for c in range(C):
        for p in range(P):
            cb[c * 8 + p, OFF_CSEL + p] = 1.0
            cb[p, OFF_CBCN + c * 8 + p] = -1.0
    for c in range(C):
        # WTa_c rows (c*16+j), cols (p*16+i) = W[p,c,i,j]
        cb[c * 16:(c + 1) * 16, OFF_WTA + c * 128:OFF_WTA + (c + 1) * 128] = \
            W[:, c].transpose(2, 0, 1).reshape(16, 128)
        for p in range(P):
            cb[p * 16:(p + 1) * 16, OFF_ASEL + c * 40 + c * 8 + p] = 1.0
            cb[c * 8 + p, OFF_BSEL + c * 128 + p * 16:
               OFF_BSEL + c * 128 + (p + 1) * 16] = 1.0
        cb[:, OFF_WF + c * 80 + c * 16:OFF_WF + c * 80 + (c + 1) * 16] = \
            W[:, c].reshape(128, 16)
    cb[:, OFF_EYE:OFF_EYE + 128] = np.eye(128, dtype=np.float32)

    import ml_dtypes
    cstb = cb.astype(ml_dtypes.bfloat16)
    return np.ascontiguousarray(cstf), np.ascontiguousarray(cstb)


def build_nc(nt: int = NT) -> bass.Bass:
    bc = nt * NB
    nc = bacc.Bacc(None)

    x_d = nc.declare_dram_parameter("xc", [bc, D], F32R, isOutput=False)
    cf_d = nc.declare_dram_parameter("cstf", [128, CF_W], F32R, isOutput=False)
    cb_d = nc.declare_dram_parameter("cstb", [128, CB_W], BF16, isOutput=False)
    v_d = nc.declare_dram_parameter("vout", [nt * 80, NB], BF16, isOutput=True)

    with tile.TileContext(nc) as tc, nc.allow_low_precision(reason="bf16 kernel"):
        with (
            tc.sbuf_pool(name="const", bufs=1) as cpool,
            tc.sbuf_pool(name="xin", bufs=2) as xpool,
            tc.sbuf_pool(name="xt", bufs=3) as xtpool,
            tc.sbuf_pool(name="mid", bufs=3) as mpool,
            tc.sbuf_pool(name="ubank", bufs=nt) as upool,
            tc.sbuf_pool(name="tmul", bufs=4) as tpool,
            tc.sbuf_pool(name="rt", bufs=4) as rtpool,
            tc.sbuf_pool(name="sm", bufs=3) as smpool,
            tc.sbuf_pool(name="out", bufs=3) as opool,
            tc.psum_pool(name="pA", bufs=2) as pA,          # pt/pu/ps/pvt
            tc.psum_pool(name="psm", bufs=1) as psm,        # small f32
            tc.psum_pool(name="pbc", bufs=3) as pbc,        # bcasts
            tc.psum_pool(name="pacc", bufs=2) as pacc,      # logits b
        ):
            # ---- constants: one DMA each, staged through DVE ----
            cf0 = cpool.tile([128, CF_W], F32R)
            nc.sync.dma_start(out=cf0[:], in_=cf_d[:])
            cf = cpool.tile([128, CF_W], F32R)
            nc.vector.tensor_copy(cf[:], cf0[:])
            cb0 = cpool.tile([128, CB_W], BF16)
            nc.sync.dma_start(out=cb0[:], in_=cb_d[:])
            cbs = cpool.tile([128, CB_W], BF16)
            nc.vector.tensor_copy(cbs[:], cb0[:])

            identf = cf[:, 0:128]
            bpf = cf[:, 128:129].bitcast(F32)

            def mm(out, lhsT, rhs, start=True, stop=True):
                nc.tensor.matmul(out, lhsT, rhs, start=start, stop=stop)

            def g_chain(pvq, rows, tag):
                """g = sqrt(s)/(1+s) = exp(0.5*ln(s) - ln(1+s))."""
                l1 = smpool.tile([rows, NB], F32, tag=f"l1{tag}")
                nc.scalar.activation(l1[:], pvq[:], AF.Ln)
                l2 = smpool.tile([rows, NB], F32, tag=f"l2{tag}")
                nc.scalar.activation(l2[:], pvq[:], AF.Ln, bias=1.0)
                gm = smpool.tile([rows, NB], F32, tag=f"gm{tag}")
                nc.vector.scalar_tensor_tensor(
                    gm[:], l1[:], 0.5, l2[:],
                    op0=AluOpType.mult, op1=AluOpType.subtract)
                g = smpool.tile([rows, NB], BF16, tag=f"g{tag}")
                nc.scalar.activation(g[:], gm[:], AF.Exp)
                return g

            u_tiles = []

            # ================= PASS 1: x -> u =================
            for it in range(nt):
                x_sb = xpool.tile([128, 4, D], F32R, tag="xin")
                src = x_d[it * NB:(it + 1) * NB, :].rearrange(
                    "(p q) d -> p q d", p=128)
                nc.sync.dma_start(out=x_sb[:], in_=src)

                xT = xtpool.tile([128, 6, NB], BF16, tag="xt")
                for k in range(6):
                    pt = pA.tile([128, NB], F32R, tag="pA")
                    for q in range(4):
                        nc.tensor.transpose(
                            pt[:, q * 128:(q + 1) * 128],
                            x_sb[:, q, k * 128:(k + 1) * 128],
                            identf)
                    nc.vector.tensor_copy(xT[:, k, :], pt[:])

                pu = pA.tile([128, NB], F32, tag="pA")
                for k in range(6):
                    mm(pu[:], cbs[:, OFF_WP + k * 128:OFF_WP + (k + 1) * 128],
                       xT[:, k, :], start=(k == 0), stop=(k == 5))
                u_pre = mpool.tile([128, NB], BF16, tag="upre")
                nc.scalar.activation(u_pre[:], pu[:], AF.Identity, bias=bpf)
                usq = mpool.tile([128, NB], BF16, tag="usq")
                nc.scalar.activation(usq[:], pu[:], AF.Square, bias=bpf)

                psq = psm.tile([8, NB], F32, tag="psm")
                mm(psq[:], cbs[:, OFF_SSEL:OFF_SSEL + 8], usq[:])
                f = g_chain(psq, 8, "f")
                pfb = pbc.tile([128, NB], F32, tag="pbc")
                mm(pfb[:], cbs[:8, OFF_SBC:OFF_SBC + 128], f[:])
                u = upool.tile([128, NB], BF16, tag="u")
                nc.vector.tensor_mul(u[:], u_pre[:], pfb[:])
                u_tiles.append(u)

            # ================= PASS 2: routing (tiles interleaved in pairs
            # so each engine always has an independent instruction ready) ===
            state = {}

            def emit_iter(it, itr):
                u = u_tiles[it]
                st = state.setdefault(it, {})
                pa = st.get("pa")

                ps = pA.tile([80, NB], F32, tag="pA")
                if itr == 0:
                    mm(ps[:], cbs[:, OFF_M0:OFF_M0 + 80], u[:])
                else:
                    e = rtpool.tile([40, NB], BF16, tag="rt_e")
                    nc.scalar.activation(e[:], pa[:], AF.Exp)
                    pden = psm.tile([8, NB], F32, tag="psm")
                    mm(pden[:], cbs[:40, OFF_CSEL:OFF_CSEL + 8], e[:])
                    lse = smpool.tile([8, NB], BF16, tag="lse")
                    nc.scalar.activation(lse[:], pden[:], AF.Ln)
                    mm(pa[:], cbs[:8, OFF_CBCN:OFF_CBCN + 40], lse[:],
                       start=False, stop=True)
                    cn = rtpool.tile([40, NB], BF16, tag="rt_cn")
                    nc.scalar.activation(cn[:], pa[:], AF.Exp)
                    t5 = tpool.tile([128, 5, NB], BF16, tag="t5")
                    for c in range(C):
                        pcb = pbc.tile([128, NB], F32, tag="pbc")
                        mm(pcb[:],
                           cbs[:40, OFF_BSEL + c * 128:
                               OFF_BSEL + (c + 1) * 128], cn[:])
                        nc.vector.tensor_mul(t5[:, c, :], u[:], pcb[:])
                    for c in range(C):
                        mm(ps[:], cbs[:, OFF_WF + c * 80:
                                      OFF_WF + (c + 1) * 80],
                           t5[:, c, :], start=(c == 0), stop=(c == 4))

                s_sb = rtpool.tile([80, NB], BF16, tag="rt_s")
                nc.scalar.copy(s_sb[:], ps[:])
                ssq = rtpool.tile([80, NB], BF16, tag="rt_ssq")
                nc.scalar.activation(ssq[:], ps[:], AF.Square)
                pvq = psm.tile([5, NB], F32, tag="psm")
                mm(pvq[:], cbs[:80, OFF_JSEL:OFF_JSEL + 5], ssq[:])
                g = g_chain(pvq, 5, "g")
                pg80 = pbc.tile([80, NB], F32, tag="pbc")
                mm(pg80[:], cbs[:5, OFF_G80:OFF_G80 + 80], g[:])
                v_sb = rtpool.tile([80, NB], BF16, tag="rt_v")
                nc.vector.tensor_mul(v_sb[:], s_sb[:], pg80[:])
                st["v"] = v_sb

                if itr < 2:
                    if itr == 0:
                        pa = pacc.tile([40, NB], F32, tag="pacc")
                        st["pa"] = pa
                    t5a = tpool.tile([128, 5, NB], BF16, tag="t5")
                    for c in range(C):
                        pwv = pbc.tile([128, NB], F32, tag="pbc")
                        mm(pwv[:],
                           cbs[:80, OFF_WTA + c * 128:
                               OFF_WTA + (c + 1) * 128], v_sb[:])
                        nc.vector.tensor_mul(t5a[:, c, :], u[:], pwv[:])
                    for c in range(C):
                        mm(pa[:], cbs[:, OFF_ASEL + c * 40:
                                      OFF_ASEL + (c + 1) * 40],
                           t5a[:, c, :],
                           start=(itr == 0 and c == 0), stop=(c == 4))

            def emit_out(it):
                v_sb = state[it]["v"]
                nc.sync.dma_start(out=v_d[it * 80:(it + 1) * 80, :],
                                  in_=v_sb[:])

            for tp in range(0, nt, 2):
                pair = [t for t in (tp, tp + 1) if t < nt]
                for itr in range(3):
                    for t in pair:
                        emit_iter(t, itr)
                for t in pair:
                    emit_out(t)

    nc.compile()
    return nc


_NC_CACHE: dict = {}


def _get_nc(nt: int) -> bass.Bass:
    if nt not in _NC_CACHE:
        _NC_CACHE[nt] = build_nc(nt)
    return _NC_CACHE[nt]


def kernel(x, Wp, bp, W):
    x = np.asarray(x, np.float32)
    cstf, cstb = build_consts(Wp, bp, W)
    nc = _get_nc(NT)
    in_maps = [{"xc": np.ascontiguousarray(x[i * BC:(i + 1) * BC]),
                "cstf": cstf, "cstb": cstb}
               for i in range(NCORES)]
    res = run_bass_kernel_spmd(nc, in_maps, list(range(NCORES)))
    parts = []
    for i in range(NCORES):
        arr = np.asarray(res.results[i]["vout"]).astype(np.float32)
        # [nt*80, 512] -> [nt, 80, 4(q), 128(p)] -> rows it*512 + p*4 + q
        arr = arr.reshape(NT, 80, 4, 128).transpose(0, 3, 2, 1).reshape(BC, 80)
        parts.append(arr)
    return np.concatenate(parts, axis=0).reshape(B, C, CD)


# revision 4
# speedup vs baseline: 1.1712x; 1.0555x over previous
"""Trainium2 Bass kernel for CapsuleLayer (nn_CapsuleLayer_45552423142009).

Two-pass structure:
  Pass 1 (dense PE stream, keeps HAM at K=8/8): per tile, DMA x,
    PE-transpose, mm1, squash -> u tile parked in SBUF (32 x 1KB/part).
  Pass 2 (routing): logits live in PSUM the whole time -- agreement
    accumulates b via start=False matmuls, and the softmax log-sum-exp
    subtraction is a negated-selector matmul into the same bank
    (softmax is shift-invariant, so the residual per-p shift cancels).
  v = s*g is computed every iteration and agreements contract v
    directly (u_hat never materialized).
  ACT uses only {Ln, Exp, Square, Identity, Copy}: one table set.
"""

import sys
import numpy as np

sys.path.insert(0, "/opt/trn_rl_repo")

from concourse import bass, bacc, mybir  # noqa: E402
from concourse import tile  # noqa: E402
from concourse.bass_utils import run_bass_kernel_spmd  # noqa: E402
from concourse.alu_op_type import AluOpType  # noqa: E402

# Pin ACT to the one table set covering every function used here
# (Ln, Exp, Square, Identity, Copy); placement otherwise first-fits
# Ln->natural_log / Exp->exp_and_others and thrashes table loads.
from concourse import hw_specs as _hw_specs  # noqa: E402

_ORIG_GAT = _hw_specs.get_activation_tables
_KEEP_SET = "natural_log_exp_and_others"


def _pinned_tables(arch):
    tabs = _ORIG_GAT(arch)
    return {k: (v if k == _KEEP_SET else set()) for k, v in tabs.items()}


_hw_specs.get_activation_tables = _pinned_tables
bacc.get_activation_tables = _pinned_tables

F32 = mybir.dt.float32
F32R = mybir.dt.float32r
BF16 = mybir.dt.bfloat16
AF = mybir.ActivationFunctionType

B = 131072
D = 768
P = 8
PD = 16
C = 5
CD = 16
NCORES = 8
BC = B // NCORES          # 16384 batch rows per core
NB = 512                  # batch columns per tile
NT = BC // NB             # 32 tiles

# bf16 const blob column offsets
OFF_WP = 0                # [128, 768]  mm1 weights, 6 chunks of [128,128]
OFF_SSEL = 768            # [128, 8]    sum 16-groups -> p
OFF_SBC = 776             # [8, 128]    broadcast p -> (p,i)
OFF_M0 = 904              # [128, 80]   0.2*W flat: s0 = M0^T u
OFF_JSEL = 984            # [80, 5]     sum j at fixed c
OFF_G80 = 1029            # [5, 80]     broadcast c -> (c,j)
OFF_CSEL = 1109           # [80, 8]     sum c at fixed p (logits at c*16+p)
OFF_CBCN = 1117           # [8, 80]     NEGATED broadcast p -> (c*16+p)
OFF_WTA = 1197            # [80, 640]   5 x [16,128] blocks: wv_c = WTa_c^T v
OFF_ASEL = 1837           # [128, 400]  5 x [128,80]: sum i -> (c*16+p)
OFF_BSEL = 2237           # [80, 640]   5 x [8,128] blocks: bcast (c*16+p)->(p,i)
OFF_WF = 2877             # [128, 400]  5 x [128,80]: s_c = Wf_c^T t_c
CB_W = OFF_WF + 400
CF_W = 129                # f32 blob: ident(128) + bp(1)


def build_consts(Wp, bp, W):
    """Return (cstf [128,129] f32, cstb [128,CB_W] bf16)."""
    Wp = np.asarray(Wp, np.float32)
    bp = np.asarray(bp, np.float32)
    W = np.asarray(W, np.float32)

    cstf = np.zeros((128, CF_W), np.float32)
    cstf[:, 0:128] = np.eye(128, dtype=np.float32)
    cstf[:, 128] = bp.reshape(128)

    cb = np.zeros((128, CB_W), np.float32)
    wp_flat = Wp.transpose(1, 0, 2).reshape(D, 128)           # [d, (p,o)]
    for k in range(6):
        cb[:, OFF_WP + k * 128:OFF_WP + (k + 1) * 128] = \
            wp_flat[k * 128:(k + 1) * 128, :]
    for p in range(P):
        for i in range(PD):
            cb[p * 16 + i, OFF_SSEL + p] = 1.0
            cb[p, OFF_SBC + p * 16 + i] = 1.0
    cb[:, OFF_M0:OFF_M0 + 80] = 0.2 * W.transpose(0, 2, 1, 3).reshape(128, 80)
    for c in range(C):
        for j in range(CD):
            cb[c * 16 + j, OFF_JSEL + c] = 1.0
            cb[c, OFF_G80 + c * 16 + j] = 1.0
    for c in range(C):
        for p in range(P):
            cb[c * 16 + p, OFF_CSEL + p] = 1.0
            cb[p, OFF_CBCN + c * 16 + p] = -1.0
    for c in range(C):
        # WTa_c rows (c*16+j), cols (p*16+i) = W[p,c,i,j]
        cb[c * 16:(c + 1) * 16, OFF_WTA + c * 128:OFF_WTA + (c + 1) * 128] = \
            W[:, c].transpose(2, 0, 1).reshape(16, 128)
        for p in range(P):
            cb[p * 16:(p + 1) * 16, OFF_ASEL + c * 80 + c * 16 + p] = 1.0
            cb[c * 16 + p, OFF_BSEL + c * 128 + p * 16:
               OFF_BSEL + c * 128 + (p + 1) * 16] = 1.0
        cb[:, OFF_WF + c * 80 + c * 16:OFF_WF + c * 80 + (c + 1) * 16] = \
            W[:, c].reshape(128, 16)

    import ml_dtypes
    cstb = cb.astype(ml_dtypes.bfloat16)
    return np.ascontiguousarray(cstf), np.ascontiguousarray(cstb)


def build_nc(nt: int = NT) -> bass.Bass:
    bc = nt * NB
    nc = bacc.Bacc(None)

    x_d = nc.declare_dram_parameter("xc", [bc, D], F32R, isOutput=False)
    cf_d = nc.declare_dram_parameter("cstf", [128, CF_W], F32R, isOutput=False)
    cb_d = nc.declare_dram_parameter("cstb", [128, CB_W], BF16, isOutput=False)
    v_d = nc.declare_dram_parameter("vout", [nt * 80, NB], BF16, isOutput=True)

    with tile.TileContext(nc) as tc, nc.allow_low_precision(reason="bf16 kernel"):
        with (
            tc.sbuf_pool(name="const", bufs=1) as cpool,
            tc.sbuf_pool(name="xin", bufs=2) as xpool,
            tc.sbuf_pool(name="xt", bufs=3) as xtpool,
            tc.sbuf_pool(name="mid", bufs=3) as mpool,
            tc.sbuf_pool(name="ubank", bufs=nt) as upool,
            tc.sbuf_pool(name="tmul", bufs=4) as tpool,
            tc.sbuf_pool(name="rt", bufs=4) as rtpool,
            tc.sbuf_pool(name="sm", bufs=3) as smpool,
            tc.sbuf_pool(name="out", bufs=3) as opool,
            tc.psum_pool(name="pA", bufs=2) as pA,          # pt/pu/ps/pvt
            tc.psum_pool(name="psm", bufs=1) as psm,        # small f32
            tc.psum_pool(name="pbc", bufs=3) as pbc,        # bcasts
            tc.psum_pool(name="pacc", bufs=2) as pacc,      # logits b
        ):
            # ---- constants: one DMA each, staged through DVE ----
            cf0 = cpool.tile([128, CF_W], F32R)
            nc.sync.dma_start(out=cf0[:], in_=cf_d[:])
            cf = cpool.tile([128, CF_W], F32R)
            nc.vector.tensor_copy(cf[:], cf0[:])
            cb0 = cpool.tile([128, CB_W], BF16)
            nc.sync.dma_start(out=cb0[:], in_=cb_d[:])
            cbs = cpool.tile([128, CB_W], BF16)
            nc.vector.tensor_copy(cbs[:], cb0[:])

            identf = cf[:, 0:128]
            bpf = cf[:, 128:129].bitcast(F32)

            def mm(out, lhsT, rhs, start=True, stop=True):
                nc.tensor.matmul(out, lhsT, rhs, start=start, stop=stop)

            def g_chain(pvq, rows, tag):
                """g = sqrt(s)/(1+s) = exp(0.5*ln(s) - ln(1+s))."""
                l1 = smpool.tile([rows, NB], F32, tag=f"l1{tag}")
                nc.scalar.activation(l1[:], pvq[:], AF.Ln)
                l2 = smpool.tile([rows, NB], F32, tag=f"l2{tag}")
                nc.scalar.activation(l2[:], pvq[:], AF.Ln, bias=1.0)
                gm = smpool.tile([rows, NB], F32, tag=f"gm{tag}")
                nc.vector.scalar_tensor_tensor(
                    gm[:], l1[:], 0.5, l2[:],
                    op0=AluOpType.mult, op1=AluOpType.subtract)
                g = smpool.tile([rows, NB], BF16, tag=f"g{tag}")
                nc.scalar.activation(g[:], gm[:], AF.Exp)
                return g

            u_tiles = []

            # ================= PASS 1: x -> u =================
            for it in range(nt):
                x_sb = xpool.tile([128, 4, D], F32R, tag="xin")
                src = x_d[it * NB:(it + 1) * NB, :].rearrange(
                    "(p q) d -> p q d", p=128)
                nc.sync.dma_start(out=x_sb[:], in_=src)

                xT = xtpool.tile([128, 6, NB], BF16, tag="xt")
                for k in range(6):
                    pt = pA.tile([128, NB], F32R, tag="pA")
                    for q in range(4):
                        nc.tensor.transpose(
                            pt[:, q * 128:(q + 1) * 128],
                            x_sb[:, q, k * 128:(k + 1) * 128],
                            identf)
                    nc.vector.tensor_copy(xT[:, k, :], pt[:])

                pu = pA.tile([128, NB], F32, tag="pA")
                for k in range(6):
                    mm(pu[:], cbs[:, OFF_WP + k * 128:OFF_WP + (k + 1) * 128],
                       xT[:, k, :], start=(k == 0), stop=(k == 5))
                u_pre = mpool.tile([128, NB], BF16, tag="upre")
                nc.scalar.activation(u_pre[:], pu[:], AF.Identity, bias=bpf)
                usq = mpool.tile([128, NB], BF16, tag="usq")
                nc.scalar.activation(usq[:], pu[:], AF.Square, bias=bpf)

                psq = psm.tile([8, NB], F32, tag="psm")
                mm(psq[:], cbs[:, OFF_SSEL:OFF_SSEL + 8], usq[:])
                f = g_chain(psq, 8, "f")
                pfb = pbc.tile([128, NB], F32, tag="pbc")
                mm(pfb[:], cbs[:8, OFF_SBC:OFF_SBC + 128], f[:])
                u = upool.tile([128, NB], BF16, tag="u")
                nc.vector.tensor_mul(u[:], u_pre[:], pfb[:])
                u_tiles.append(u)

            # ================= PASS 2: routing (tiles interleaved in pairs
            # so each engine always has an independent instruction ready) ===
            state = {}

            def emit_iter(it, itr):
                u = u_tiles[it]
                st = state.setdefault(it, {})
                pa = st.get("pa")

                ps = pA.tile([80, NB], F32, tag="pA")
                if itr == 0:
                    mm(ps[:], cbs[:, OFF_M0:OFF_M0 + 80], u[:])
                else:
                    e = rtpool.tile([80, NB], BF16, tag="rt_e")
                    nc.scalar.activation(e[:], pa[:], AF.Exp)
                    pden = psm.tile([8, NB], F32, tag="psm")
                    mm(pden[:], cbs[:80, OFF_CSEL:OFF_CSEL + 8], e[:])
                    lse = smpool.tile([8, NB], BF16, tag="lse")
                    nc.scalar.activation(lse[:], pden[:], AF.Ln)
                    mm(pa[:], cbs[:8, OFF_CBCN:OFF_CBCN + 80], lse[:],
                       start=False, stop=True)
                    cn = rtpool.tile([80, NB], BF16, tag="rt_cn")
                    nc.scalar.activation(cn[:], pa[:], AF.Exp)
                    t5 = tpool.tile([128, 5, NB], BF16, tag="t5")
                    # classes 0/2/4 contract only their 8 live rows, at
                    # 32-aligned bases -> concurrent PE row-group tiles
                    for c in (0, 2, 4, 1, 3):
                        pcb = pbc.tile([128, NB], F32, tag="pbc")
                        if c in (0, 2, 4):
                            mm(pcb[:],
                               cbs[c * 16:c * 16 + 8,
                                   OFF_BSEL + c * 128:
                                   OFF_BSEL + (c + 1) * 128],
                               cn[c * 16:c * 16 + 8, :])
                        else:
                            mm(pcb[:],
                               cbs[:80, OFF_BSEL + c * 128:
                                   OFF_BSEL + (c + 1) * 128], cn[:])
                        nc.vector.tensor_mul(t5[:, c, :], u[:], pcb[:])
                    for ci, c in enumerate((0, 2, 4, 1, 3)):
                        mm(ps[:], cbs[:, OFF_WF + c * 80:
                                      OFF_WF + (c + 1) * 80],
                           t5[:, c, :], start=(ci == 0), stop=(ci == 4))

                s_sb = rtpool.tile([80, NB], BF16, tag="rt_s")
                nc.scalar.copy(s_sb[:], ps[:])
                ssq = rtpool.tile([80, NB], BF16, tag="rt_ssq")
                nc.scalar.activation(ssq[:], ps[:], AF.Square)
                pvq = psm.tile([5, NB], F32, tag="psm")
                mm(pvq[:], cbs[:80, OFF_JSEL:OFF_JSEL + 5], ssq[:])
                g = g_chain(pvq, 5, "g")
                pg80 = pbc.tile([80, NB], F32, tag="pbc")
                mm(pg80[:], cbs[:5, OFF_G80:OFF_G80 + 80], g[:])
                v_sb = rtpool.tile([80, NB], BF16, tag="rt_v")
                nc.vector.tensor_mul(v_sb[:], s_sb[:], pg80[:])
                st["v"] = v_sb

                if itr < 2:
                    if itr == 0:
                        pa = pacc.tile([80, NB], F32, tag="pacc")
                        st["pa"] = pa
                    t5a = tpool.tile([128, 5, NB], BF16, tag="t5")
                    for c in (0, 2, 4, 1, 3):
                        pwv = pbc.tile([128, NB], F32, tag="pbc")
                        if c in (0, 2, 4):
                            mm(pwv[:],
                               cbs[c * 16:(c + 1) * 16,
                                   OFF_WTA + c * 128:
                                   OFF_WTA + (c + 1) * 128],
                               v_sb[c * 16:(c + 1) * 16, :])
                        else:
                            mm(pwv[:],
                               cbs[:80, OFF_WTA + c * 128:
                                   OFF_WTA + (c + 1) * 128], v_sb[:])
                        nc.vector.tensor_mul(t5a[:, c, :], u[:], pwv[:])
                    for ci, c in enumerate((0, 2, 4, 1, 3)):
                        mm(pa[:], cbs[:, OFF_ASEL + c * 80:
                                      OFF_ASEL + (c + 1) * 80],
                           t5a[:, c, :],
                           start=(itr == 0 and ci == 0), stop=(ci == 4))

            def emit_out(it):
                v_sb = state[it]["v"]
                nc.sync.dma_start(out=v_d[it * 80:(it + 1) * 80, :],
                                  in_=v_sb[:])

            for tp in range(0, nt, 2):
                pair = [t for t in (tp, tp + 1) if t < nt]
                for itr in range(3):
                    for t in pair:
                        emit_iter(t, itr)
                for t in pair:
                    emit_out(t)

    nc.compile()
    return nc


_NC_CACHE: dict = {}


def _get_nc(nt: int) -> bass.Bass:
    if nt not in _NC_CACHE:
        _NC_CACHE[nt] = build_nc(nt)
    return _NC_CACHE[nt]


def kernel(x, Wp, bp, W):
    x = np.asarray(x, np.float32)
    cstf, cstb = build_consts(Wp, bp, W)
    nc = _get_nc(NT)
    in_maps = [{"xc": np.ascontiguousarray(x[i * BC:(i + 1) * BC]),
                "cstf": cstf, "cstb": cstb}
               for i in range(NCORES)]
    res = run_bass_kernel_spmd(nc, in_maps, list(range(NCORES)))
    parts = []
    for i in range(NCORES):
        arr = np.asarray(res.results[i]["vout"]).astype(np.float32)
        # [nt*80, 512] -> [nt, 80, 4(q), 128(p)] -> rows it*512 + p*4 + q
        arr = arr.reshape(NT, 80, 4, 128).transpose(0, 3, 2, 1).reshape(BC, 80)
        parts.append(arr)
    return np.concatenate(parts, axis=0).reshape(B, C, CD)
